# revision 23
# baseline (speedup 1.0000x reference)
"""Trainium2 Bass kernel for nn_EyeRobotAgent block-sparse ("eye") attention.

Shapes: q,k,v [2, 12, 3456, 32] fp32.  S = 16 time-blocks x 216 feats.
Mask structure (per query block t):
  - all 216 keys of block t are candidates (minus img->img),
  - of each past block t-7..t-1, only 19 keys (m in {0..3, 5..19}) are
    visible (proprio m==4 and img m>=20 keys are never visible in the past),
  - joint queries (m in [4,20)) cannot see past joint keys,
  - img queries (m >= 20) cannot see img keys at all.

Strategy (data-parallel: 24 (b,h) pairs over 8 cores, 3 each).
Sparsity-aware score layout: img queries (196 of 216 per block) only see
153 kv (133 past + 20 same-block non-img), small queries (m 0..19) see
349.  Scores are computed transposed [kv, q] in 128-partition-exact
chunks, grouped 4 blocks (2 pairs) per PSUM tile so ONE exp() ACT op
covers 1240 columns.  Masking: joint-past via one augmented contraction
row (row32); invalid/pad kv need no mask at all because their V rows
and ones-column are zero (they contribute 0 to both numerator and
denominator).  32-row kv chunks stripe 4 blocks into one 128-partition
bank.  PV consumes probs as the stationary operand giving out [q, 33]
directly; normalization (reciprocal+mul) runs on DVE from PSUM.  The
DRAM output is a partition-major fp16 scratch layout; the host scatters
it back to [S, D] fp32 (free).
"""
import numpy as np

import concourse.bass as bass
import concourse.mybir as mybir
import concourse.tile as tile
from concourse import bacc
from concourse.bass_utils import run_bass_kernel_spmd

B, H, S, D = 2, 12, 3456, 32
F = 216            # feats_per_t
W = 8              # window_len
T = S // F         # 16 blocks
IMG_START = 20     # F - img_feat_size
NIMG = F - IMG_START   # 196 img queries per block
PAST_SEL = np.array([0, 1, 2, 3] + list(range(5, 20)))   # 19 per past block
NPAST = 19 * (W - 1)     # 133
KBLK = 356               # kall cols/block: 133 past |3 pad| 20 |4 pad| 196
VA = D + 1               # 33 = v columns + ones column
NEG = np.float32(-30000.0)
SCALE = float(1.0 / np.sqrt(np.float32(D)))
N_CORES = 8
BH_PER_CORE = (B * H) // N_CORES      # 3
NGRP = T // 4                         # 4 groups of 4 blocks per (b,h)

F32 = mybir.dt.float32
FP16 = mybir.dt.float16
NP_FP16 = np.float16

# scores col layout per 4-block group: 3 PSUM banks (512 fp32 cols each),
# every matmul output region within one bank, zero column gaps (1240 cols).
# Group 0 (blocks 0..3): block 0 has no valid past keys, so its img-c0 and
# sm-c0 chunks are skipped entirely -> compact 1024-col (2 bank) map.
CM = {
    "img0": (0, 196, 512, 708),      # [j] img q x past[0:128]
    "img1": 1024,                    # img q x c1-32 (4 x 32-row stripes)
    "sm0": (392, 412, 432, 452),     # [j] small q x past[0:128]
    "sm1": 472,                      # small q x c1-32 (stripes)
    "sm2": (492, 904, 924, 944),     # [j] small q x same m20..147
    "sm3": (964, 984, 1004, 1220),   # [j] small q x same m148..215
    "ncol": 1240, "skip0": False,
}
CM0 = {
    "img0": (None, 0, 196, 512),
    "img1": 708,
    "sm0": (None, 392, 412, 432),
    "sm1": 452,
    "sm2": (472, 492, 904, 924),
    "sm3": (944, 964, 984, 1004),
    "ncol": 1024, "skip0": True,
}
NCOL = 1240


# ---------------------------------------------------------------- host packing

def _pack_all(q, k, v):
    nbh = B * H
    qf = q.reshape(nbh, S, D).astype(np.float32)
    kf = k.reshape(nbh, S, D).astype(np.float32)
    vf = v.reshape(nbh, S, D).astype(np.float32)

    is_joint = lambda m: (m >= 4) & (m < IMG_START)

    # qaug [nbh, 33, S]: rows 0..31 q^T, row32 = is_joint(s % F)
    qaug = np.empty((nbh, 33, S), np.float32)
    qaug[:, :D] = qf.transpose(0, 2, 1)
    qaug[:, 32] = is_joint(np.arange(S) % F).astype(np.float32)

    # kall [nbh, 33, 16*356 + 60]
    kall = np.zeros((nbh, 33, T * KBLK + 60), np.float32)
    # vall [nbh, 128, T, 4, 33]
    vall = np.zeros((nbh, 128, T, 4, VA), np.float32)

    sel_m = np.tile(PAST_SEL, W - 1)                      # [133] m of past idx
    sel_tau_off = np.repeat(np.arange(-7, 0), 19)         # [133] tau - t
    joint_bias = NEG * is_joint(sel_m).astype(np.float32)  # [133]

    for t in range(T):
        base = KBLK * t
        taus = t + sel_tau_off
        valid = taus >= 0
        rows = np.where(valid, F * taus + sel_m, 0)
        kpast = np.where(valid[None, None, :], kf[:, rows].transpose(0, 2, 1), 0.0)
        # past cols
        kall[:, :D, base:base + NPAST] = kpast
        kall[:, 32, base:base + NPAST] = joint_bias
        # nonimg 20 (m 0..19 of block t) at cols base+136..155
        kall[:, :D, base + 136:base + 156] = \
            kf[:, F * t:F * t + IMG_START].transpose(0, 2, 1)
        # same-img 196 at cols base+160..355
        kall[:, :D, base + 160:base + KBLK] = \
            kf[:, F * t + IMG_START:F * (t + 1)].transpose(0, 2, 1)

        vpast = np.where(valid[None, :, None], vf[:, rows], 0.0)  # [nbh,133,32]
        ones_v = valid.astype(np.float32)
        # c0: past idx 0..127
        vall[:, :, t, 0, :D] = vpast[:, :128]
        vall[:, :, t, 0, 32] = ones_v[:128]
        # c1: stripe at partitions 32*j: [past 128:133 |0x3| m0..19 |0x4]
        j = t % 4
        sb = 32 * j
        sl = slice(sb, sb + 5)
        vall[:, sl, t, 1, :D] = vpast[:, 128:133]
        vall[:, sl, t, 1, 32] = ones_v[128:133]
        sl2 = slice(sb + 8, sb + 28)
        vall[:, sl2, t, 1, :D] = vf[:, F * t:F * t + IMG_START]
        vall[:, sl2, t, 1, 32] = 1.0
        # c2: same m20..147
        vall[:, :, t, 2, :D] = vf[:, F * t + 20:F * t + 148]
        vall[:, :, t, 2, 32] = 1.0
        # c3: same m148..215 at partitions 0..67
        vall[:, :68, t, 3, :D] = vf[:, F * t + 148:F * (t + 1)]
        vall[:, :68, t, 3, 32] = 1.0

    return {"qaug": qaug.astype(NP_FP16),
            "kall": kall.astype(NP_FP16),
            "vall": np.ascontiguousarray(vall).astype(NP_FP16)}


def _unpack_out(r0, r1, r2):
    """r0 [nbh,128,T,32], r1 [nbh,68,T,32], r2 [nbh,20,T,32] fp16 -> fp32"""
    nbh = r0.shape[0]
    out = np.empty((nbh, S, D), np.float32)
    for t in range(T):
        out[:, F * t + 20:F * t + 148] = r0[:, :, t]
        out[:, F * t + 148:F * (t + 1)] = r1[:, :, t]
        out[:, F * t:F * t + 20] = r2[:, :, t]
    return out


# ---------------------------------------------------------------- bass kernel

def build_nc(n_bh=BH_PER_CORE):
    nc = bacc.Bacc(None, target_bir_lowering=False, debug=False)
    qaug_d = nc.declare_dram_parameter("qaug", [BH_PER_CORE, 33, S], FP16, isOutput=False)
    kall_d = nc.declare_dram_parameter("kall", [BH_PER_CORE, 33, T * KBLK + 60], FP16, isOutput=False)
    vall_d = nc.declare_dram_parameter("vall", [BH_PER_CORE, 128, T, 4, VA], FP16, isOutput=False)
    out_d0 = nc.declare_dram_parameter("out0", [BH_PER_CORE, 128, T, D], FP16, isOutput=True)
    out_d1 = nc.declare_dram_parameter("out1", [BH_PER_CORE, 68, T, D], FP16, isOutput=True)
    out_d2 = nc.declare_dram_parameter("out2", [BH_PER_CORE, 20, T, D], FP16, isOutput=True)

    def _str2(ap, d1, d2):
        return bass.AP(tensor=ap.tensor, offset=ap.offset,
                       ap=[list(ap.ap[0]), list(d1), list(d2)])

    with tile.TileContext(nc) as tc:
        with (
            tc.tile_pool(name="qp", bufs=2) as qp,
            tc.tile_pool(name="kp", bufs=2) as kp,
            tc.tile_pool(name="vp", bufs=2) as vp,
            tc.tile_pool(name="probsp", bufs=4) as probsp,
            tc.tile_pool(name="recipsp", bufs=3) as recipsp,
            tc.tile_pool(name="outsbp", bufs=2) as outsbp,
            tc.tile_pool(name="scoresp", bufs=2, space="PSUM") as scoresp,
            tc.tile_pool(name="pvp", bufs=2, space="PSUM") as pvp,
        ):
            # warm the Exp activation table while the first loads run
            scratch = qp.tile([1, 4], F32)
            nc.gpsimd.memset(scratch[:], 0.0)
            nc.scalar.activation(scratch[:], scratch[:],
                                 mybir.ActivationFunctionType.Exp, scale=1.0)

            K4 = 4 * KBLK
            def do_qk_exp(q_sb, k_sb, g):
                cm = CM0 if g == 0 else CM
                sc = scoresp.tile([128, 1536], F32)
                for j in range(4):
                    t = 4 * g + j
                    kb = KBLK * t
                    sb = 32 * j
                    qi = q_sb[0:32, F * t + 20:F * (t + 1)]
                    qs = q_sb[0:33, F * t:F * t + 20]
                    if cm["img0"][j] is not None:
                        nc.tensor.matmul(
                            sc[0:128, cm["img0"][j]:cm["img0"][j] + NIMG],
                            lhsT=k_sb[0:32, kb:kb + 128], rhs=qi,
                            start=True, stop=True)
                        nc.tensor.matmul(
                            sc[0:128, cm["sm0"][j]:cm["sm0"][j] + 20],
                            lhsT=k_sb[0:33, kb:kb + 128], rhs=qs,
                            start=True, stop=True)
                    nc.tensor.matmul(
                        sc[sb:sb + 32, cm["img1"]:cm["img1"] + NIMG],
                        lhsT=k_sb[0:32, kb + 128:kb + 160], rhs=qi,
                        start=True, stop=True, tile_position=(0, sb))
                    nc.tensor.matmul(
                        sc[sb:sb + 32, cm["sm1"]:cm["sm1"] + 20],
                        lhsT=k_sb[0:33, kb + 128:kb + 160], rhs=qs,
                        start=True, stop=True, tile_position=(0, sb))
                    nc.tensor.matmul(
                        sc[0:128, cm["sm2"][j]:cm["sm2"][j] + 20],
                        lhsT=k_sb[0:33, kb + 160:kb + 288], rhs=qs,
                        start=True, stop=True)
                    nc.tensor.matmul(
                        sc[0:128, cm["sm3"][j]:cm["sm3"][j] + 20],
                        lhsT=k_sb[0:33, kb + 288:kb + 416], rhs=qs,
                        start=True, stop=True)
                probs = probsp.tile([128, NCOL], FP16)
                nc.scalar.activation(probs[0:128, 0:cm["ncol"]],
                                     sc[0:128, 0:cm["ncol"]],
                                     mybir.ActivationFunctionType.Exp,
                                     scale=SCALE)
                return probs

            def do_pv(probs, v_sb, g):
                cm = CM0 if g == 0 else CM
                pv = pvp.tile([128, 12, VA], F32)
                for j in range(4):
                    t = 4 * g + j
                    sb = 32 * j
                    st = slice(sb, sb + 32)
                    has0 = cm["img0"][j] is not None
                    for sub, off, w in ((0, 0, 128), (1, 128, 68)):
                        if has0:
                            nc.tensor.matmul(
                                pv[0:w, 3 * j + sub, :],
                                lhsT=probs[0:128, cm["img0"][j] + off:
                                           cm["img0"][j] + off + w],
                                rhs=v_sb[0:128, t, 0, :],
                                start=True, stop=False)
                        nc.tensor.matmul(
                            pv[0:w, 3 * j + sub, :],
                            lhsT=probs[st, cm["img1"] + off:cm["img1"] + off + w],
                            rhs=v_sb[st, t, 1, :],
                            start=not has0, stop=True, tile_position=(sb, 0))
                    chunks = [(cm["sm1"], 1, st, (sb, 0)),
                              (cm["sm2"][j], 2, slice(0, 128), None),
                              (cm["sm3"][j], 3, slice(0, 128), None)]
                    if has0:
                        chunks.insert(0, (cm["sm0"][j], 0, slice(0, 128), None))
                    for ci, (col, vc, kpart, tp) in enumerate(chunks):
                        nc.tensor.matmul(
                            pv[0:20, 3 * j + 2, :],
                            lhsT=probs[kpart, col:col + 20],
                            rhs=v_sb[kpart, t, vc, :],
                            start=(ci == 0), stop=(ci == len(chunks) - 1),
                            tile_position=tp)

                return pv

            def do_norm(pv, outs, g, fine=False):
                o0, o1, o2 = outs
                recips = recipsp.tile([128, 112], F32)
                pvf = pv[:].rearrange("p g x -> p (g x)")
                nj = 4 if fine else 1
                step = 4 // nj
                for jj in range(nj):
                    c0 = 99 * step * jj
                    nc.vector.reciprocal(
                        recips[0:128, 4 * jj:4 * jj + step],
                        _str2(pvf[0:128, c0 + 32:c0 + 33], (99, step), (1, 1)))
                    nc.vector.reciprocal(
                        recips[0:68, 48 + 4 * jj:48 + 4 * jj + step],
                        _str2(pvf[0:68, c0 + 65:c0 + 66], (99, step), (1, 1)))
                    nc.vector.reciprocal(
                        recips[0:20, 96 + 4 * jj:96 + 4 * jj + step],
                        _str2(pvf[0:20, c0 + 98:c0 + 99], (99, step), (1, 1)))
                    for (ot, phi, sub) in ((o0, 128, 0), (o1, 68, 1), (o2, 20, 2)):
                        num_v = _str2(pvf[0:phi, c0 + 33 * sub:c0 + 33 * sub + 1],
                                      (99, step), (1, 32))
                        rec_b = _str2(recips[0:phi, 48 * sub + 4 * jj:
                                             48 * sub + 4 * jj + 1],
                                      (1, step), (0, 32))
                        out_v = _str2(ot[0:phi, 4 * g + step * jj, 0:1],
                                      (D, step), (1, 32))
                        nc.vector.tensor_mul(out_v, num_v, rec_b)

            # warm the Exp activation table while the first loads run
            scratch = qp.tile([1, 4], F32)
            nc.gpsimd.memset(scratch[:], 0.0)
            nc.scalar.activation(scratch[:], scratch[:],
                                 mybir.ActivationFunctionType.Exp, scale=1.0)

            K4 = 4 * KBLK
            pending = None   # (probs, v_sb, outs, g, i_done)
            for i in range(n_bh):
                q_sb = qp.tile([33, S], FP16)
                k_sb = kp.tile([33, T * KBLK + 60], FP16)
                v_sb = vp.tile([128, T, 4, VA], FP16)
                kc = [0, K4 + 60, 2 * K4 + 60, 3 * K4 + 60, 4 * K4 + 60]
                for g in range(4):
                    if i == 0 and g == 0:
                        # first chunk: parallelize across SP/Pool/ACT queues
                        half = (kc[1] // 2) & ~1
                        nc.sync.dma_start(out=k_sb[:, 0:half],
                                          in_=kall_d[0, :, 0:half])
                        nc.gpsimd.dma_start(out=k_sb[:, half:kc[1]],
                                            in_=kall_d[0, :, half:kc[1]])
                        nc.scalar.dma_start(out=q_sb[:, 0:864],
                                            in_=qaug_d[0, :, 0:864])
                    else:
                        nc.sync.dma_start(out=k_sb[:, kc[g]:kc[g + 1]],
                                          in_=kall_d[i, :, kc[g]:kc[g + 1]])
                        nc.gpsimd.dma_start(
                            out=q_sb[:, 864 * g:864 * (g + 1)],
                            in_=qaug_d[i, :, 864 * g:864 * (g + 1)])
                for g in (0, 2):
                    nc.gpsimd.dma_start(
                        out=v_sb[:, 4 * g:4 * g + 8, :, :],
                        in_=vall_d[i, :, 4 * g:4 * g + 8, :, :])
                o0 = outsbp.tile([128, T, D], FP16)
                o1 = outsbp.tile([68, T, D], FP16)
                o2 = outsbp.tile([20, T, D], FP16)

                for g in range(NGRP):
                    probs = do_qk_exp(q_sb, k_sb, g)
                    if pending is not None:
                        pvt = do_pv(pending[0], pending[1], pending[3])
                        do_norm(pvt, pending[2], pending[3])
                        if pending[4] is not None:
                            st_i, st_o = pending[4]
                            nc.sync.dma_start(out=out_d0[st_i], in_=st_o[0][:])
                            nc.gpsimd.dma_start(out=out_d1[st_i], in_=st_o[1][:])
                            nc.gpsimd.dma_start(out=out_d2[st_i], in_=st_o[2][:])
                    pending = (probs, v_sb, (o0, o1, o2), g,
                               (i, (o0, o1, o2)) if g == NGRP - 1 else None)

            pvt = do_pv(pending[0], pending[1], pending[3])
            do_norm(pvt, pending[2], pending[3])
            st_i, st_o = pending[4]
            nc.sync.dma_start(out=out_d0[st_i], in_=st_o[0][:])
            nc.gpsimd.dma_start(out=out_d1[st_i], in_=st_o[1][:])
            nc.scalar.dma_start(out=out_d2[st_i], in_=st_o[2][:])
    nc.compile()
    return nc


_NC = None


def _get_nc():
    global _NC
    if _NC is None:
        _NC = build_nc()
    return _NC


# ---------------------------------------------------------------- entry point

def kernel(q, k, v, feats_per_t, window_len, act_size, img_feat_size):
    assert int(feats_per_t) == F and int(window_len) == W
    assert int(act_size) == 16 and int(img_feat_size) == 196

    packed = _pack_all(np.asarray(q, np.float32), np.asarray(k, np.float32),
                       np.asarray(v, np.float32))
    in_maps = []
    for core in range(N_CORES):
        s = slice(BH_PER_CORE * core, BH_PER_CORE * (core + 1))
        in_maps.append({n: np.ascontiguousarray(a[s])
                        for n, a in packed.items()})

    nc = _get_nc()
    res = run_bass_kernel_spmd(nc, in_maps, list(range(N_CORES)))
    out = np.empty((B * H, S, D), np.float32)
    for core in range(N_CORES):
        r = res.results[core]
        o = _unpack_out(r["out0"].astype(np.float32),
                        r["out1"].astype(np.float32),
                        r["out2"].astype(np.float32))
        out[BH_PER_CORE * core:BH_PER_CORE * (core + 1)] = o
    return out.reshape(B, H, S, D)


# revision 24
# speedup vs baseline: 1.0188x; 1.0188x over previous
"""Trainium2 Bass kernel for nn_EyeRobotAgent block-sparse ("eye") attention.

Shapes: q,k,v [2, 12, 3456, 32] fp32.  S = 16 time-blocks x 216 feats.
Mask structure (per query block t):
  - all 216 keys of block t are candidates (minus img->img),
  - of each past block t-7..t-1, only 19 keys (m in {0..3, 5..19}) are
    visible (proprio m==4 and img m>=20 keys are never visible in the past),
  - joint queries (m in [4,20)) cannot see past joint keys,
  - img queries (m >= 20) cannot see img keys at all.

Strategy (data-parallel: 24 (b,h) pairs over 8 cores, 3 each).
Sparsity-aware score layout: img queries (196 of 216 per block) only see
153 kv (133 past + 20 same-block non-img), small queries (m 0..19) see
349.  Scores are computed transposed [kv, q] in 128-partition-exact
chunks, grouped 4 blocks (2 pairs) per PSUM tile so ONE exp() ACT op
covers 1240 columns.  Masking: joint-past via one augmented contraction
row (row32); invalid/pad kv need no mask at all because their V rows
and ones-column are zero (they contribute 0 to both numerator and
denominator).  32-row kv chunks stripe 4 blocks into one 128-partition
bank.  PV consumes probs as the stationary operand giving out [q, 33]
directly; normalization (reciprocal+mul) runs on DVE from PSUM.  The
DRAM output is a partition-major fp16 scratch layout; the host scatters
it back to [S, D] fp32 (free).
"""
import numpy as np

import concourse.bass as bass
import concourse.mybir as mybir
import concourse.tile as tile
from concourse import bacc
from concourse.bass_utils import run_bass_kernel_spmd

B, H, S, D = 2, 12, 3456, 32
F = 216            # feats_per_t
W = 8              # window_len
T = S // F         # 16 blocks
IMG_START = 20     # F - img_feat_size
NIMG = F - IMG_START   # 196 img queries per block
PAST_SEL = np.array([0, 1, 2, 3] + list(range(5, 20)))   # 19 per past block
NPAST = 19 * (W - 1)     # 133
KBLK = 356               # kall cols/block: 133 past |3 pad| 20 |4 pad| 196
VA = D + 1               # 33 = v columns + ones column
NEG = np.float32(-30000.0)
SCALE = float(1.0 / np.sqrt(np.float32(D)))
N_CORES = 8
BH_PER_CORE = (B * H) // N_CORES      # 3
NGRP = T // 4                         # 4 groups of 4 blocks per (b,h)

F32 = mybir.dt.float32
FP16 = mybir.dt.float16
NP_FP16 = np.float16

# scores col layout per 4-block group: 3 PSUM banks (512 fp32 cols each),
# every matmul output region within one bank, zero column gaps (1240 cols).
# Group 0 (blocks 0..3): block 0 has no valid past keys, so its img-c0 and
# sm-c0 chunks are skipped entirely -> compact 1024-col (2 bank) map.
CM = {
    "img0": (0, 196, 512, 708),      # [j] img q x past[0:128]
    "img1": 1024,                    # img q x c1-32 (4 x 32-row stripes)
    "sm0": (392, 412, 432, 452),     # [j] small q x past[0:128]
    "sm1": 472,                      # small q x c1-32 (stripes)
    "sm2": (492, 904, 924, 944),     # [j] small q x same m20..147
    "sm3": (964, 984, 1004, 1220),   # [j] small q x same m148..215
    "ncol": 1240, "skip0": False,
}
CM0 = {
    "img0": (None, 0, 196, 512),
    "img1": 708,
    "sm0": (None, 392, 412, 432),
    "sm1": 452,
    "sm2": (472, 492, 904, 924),
    "sm3": (944, 964, 984, 1004),
    "ncol": 1024, "skip0": True,
}
NCOL = 1240


# ---------------------------------------------------------------- host packing

def _pack_all(q, k, v):
    nbh = B * H
    qf = q.reshape(nbh, S, D).astype(np.float32)
    kf = k.reshape(nbh, S, D).astype(np.float32)
    vf = v.reshape(nbh, S, D).astype(np.float32)

    is_joint = lambda m: (m >= 4) & (m < IMG_START)

    # qaug [nbh, 33, S]: rows 0..31 q^T, row32 = is_joint(s % F)
    qaug = np.empty((nbh, 33, S), np.float32)
    qaug[:, :D] = qf.transpose(0, 2, 1)
    qaug[:, 32] = is_joint(np.arange(S) % F).astype(np.float32)

    # kall [nbh, 33, 16*356 + 60]
    kall = np.zeros((nbh, 33, T * KBLK + 60), np.float32)
    # vall [nbh, 128, T, 4, 33]
    vall = np.zeros((nbh, 128, T, 4, VA), np.float32)

    sel_m = np.tile(PAST_SEL, W - 1)                      # [133] m of past idx
    sel_tau_off = np.repeat(np.arange(-7, 0), 19)         # [133] tau - t
    joint_bias = NEG * is_joint(sel_m).astype(np.float32)  # [133]

    for t in range(T):
        base = KBLK * t
        taus = t + sel_tau_off
        valid = taus >= 0
        rows = np.where(valid, F * taus + sel_m, 0)
        kpast = np.where(valid[None, None, :], kf[:, rows].transpose(0, 2, 1), 0.0)
        # past cols
        kall[:, :D, base:base + NPAST] = kpast
        kall[:, 32, base:base + NPAST] = joint_bias
        # nonimg 20 (m 0..19 of block t) at cols base+136..155
        kall[:, :D, base + 136:base + 156] = \
            kf[:, F * t:F * t + IMG_START].transpose(0, 2, 1)
        # same-img 196 at cols base+160..355
        kall[:, :D, base + 160:base + KBLK] = \
            kf[:, F * t + IMG_START:F * (t + 1)].transpose(0, 2, 1)

        vpast = np.where(valid[None, :, None], vf[:, rows], 0.0)  # [nbh,133,32]
        ones_v = valid.astype(np.float32)
        # c0: past idx 0..127
        vall[:, :, t, 0, :D] = vpast[:, :128]
        vall[:, :, t, 0, 32] = ones_v[:128]
        # c1: stripe at partitions 32*j: [past 128:133 |0x3| m0..19 |0x4]
        j = t % 4
        sb = 32 * j
        sl = slice(sb, sb + 5)
        vall[:, sl, t, 1, :D] = vpast[:, 128:133]
        vall[:, sl, t, 1, 32] = ones_v[128:133]
        sl2 = slice(sb + 8, sb + 28)
        vall[:, sl2, t, 1, :D] = vf[:, F * t:F * t + IMG_START]
        vall[:, sl2, t, 1, 32] = 1.0
        # c2: same m20..147
        vall[:, :, t, 2, :D] = vf[:, F * t + 20:F * t + 148]
        vall[:, :, t, 2, 32] = 1.0
        # c3: same m148..215 at partitions 0..67
        vall[:, :68, t, 3, :D] = vf[:, F * t + 148:F * (t + 1)]
        vall[:, :68, t, 3, 32] = 1.0

    return {"qaug": qaug.astype(NP_FP16),
            "kall": kall.astype(NP_FP16),
            "vall": np.ascontiguousarray(vall).astype(NP_FP16)}


def _unpack_out(r0, r1, r2):
    """r0 [nbh,128,T,32], r1 [nbh,68,T,32], r2 [nbh,20,T,32] fp16 -> fp32"""
    nbh = r0.shape[0]
    out = np.empty((nbh, S, D), np.float32)
    for t in range(T):
        out[:, F * t + 20:F * t + 148] = r0[:, :, t]
        out[:, F * t + 148:F * (t + 1)] = r1[:, :, t]
        out[:, F * t:F * t + 20] = r2[:, :, t]
    return out


# ---------------------------------------------------------------- bass kernel

def build_nc(n_bh=BH_PER_CORE):
    nc = bacc.Bacc(None, target_bir_lowering=False, debug=False)
    qaug_d = nc.declare_dram_parameter("qaug", [BH_PER_CORE, 33, S], FP16, isOutput=False)
    kall_d = nc.declare_dram_parameter("kall", [BH_PER_CORE, 33, T * KBLK + 60], FP16, isOutput=False)
    vall_d = nc.declare_dram_parameter("vall", [BH_PER_CORE, 128, T, 4, VA], FP16, isOutput=False)
    out_d0 = nc.declare_dram_parameter("out0", [BH_PER_CORE, 128, T, D], FP16, isOutput=True)
    out_d1 = nc.declare_dram_parameter("out1", [BH_PER_CORE, 68, T, D], FP16, isOutput=True)
    out_d2 = nc.declare_dram_parameter("out2", [BH_PER_CORE, 20, T, D], FP16, isOutput=True)

    def _str2(ap, d1, d2):
        return bass.AP(tensor=ap.tensor, offset=ap.offset,
                       ap=[list(ap.ap[0]), list(d1), list(d2)])

    with tile.TileContext(nc) as tc:
        with (
            tc.tile_pool(name="qp", bufs=2) as qp,
            tc.tile_pool(name="kp", bufs=2) as kp,
            tc.tile_pool(name="vp", bufs=2) as vp,
            tc.tile_pool(name="probsp", bufs=4) as probsp,
            tc.tile_pool(name="recipsp", bufs=3) as recipsp,
            tc.tile_pool(name="outsbp", bufs=2) as outsbp,
            tc.tile_pool(name="scoresp", bufs=2, space="PSUM") as scoresp,
            tc.tile_pool(name="pvp", bufs=2, space="PSUM") as pvp,
        ):
            # warm the Exp activation table while the first loads run
            scratch = qp.tile([1, 4], F32)
            nc.gpsimd.memset(scratch[:], 0.0)
            nc.scalar.activation(scratch[:], scratch[:],
                                 mybir.ActivationFunctionType.Exp, scale=1.0)

            K4 = 4 * KBLK
            def do_qk_exp(q_sb, k_sb, g):
                cm = CM0 if g == 0 else CM
                sc = scoresp.tile([128, 1536], F32)
                for j in range(4):
                    t = 4 * g + j
                    kb = KBLK * t
                    sb = 32 * j
                    qi = q_sb[0:32, F * t + 20:F * (t + 1)]
                    qs = q_sb[0:33, F * t:F * t + 20]
                    if cm["img0"][j] is not None:
                        nc.tensor.matmul(
                            sc[0:128, cm["img0"][j]:cm["img0"][j] + NIMG],
                            lhsT=k_sb[0:32, kb:kb + 128], rhs=qi,
                            start=True, stop=True)
                        nc.tensor.matmul(
                            sc[0:128, cm["sm0"][j]:cm["sm0"][j] + 20],
                            lhsT=k_sb[0:33, kb:kb + 128], rhs=qs,
                            start=True, stop=True)
                    nc.tensor.matmul(
                        sc[sb:sb + 32, cm["img1"]:cm["img1"] + NIMG],
                        lhsT=k_sb[0:32, kb + 128:kb + 160], rhs=qi,
                        start=True, stop=True, tile_position=(0, sb))
                    nc.tensor.matmul(
                        sc[sb:sb + 32, cm["sm1"]:cm["sm1"] + 20],
                        lhsT=k_sb[0:33, kb + 128:kb + 160], rhs=qs,
                        start=True, stop=True, tile_position=(0, sb))
                    nc.tensor.matmul(
                        sc[0:128, cm["sm2"][j]:cm["sm2"][j] + 20],
                        lhsT=k_sb[0:33, kb + 160:kb + 288], rhs=qs,
                        start=True, stop=True)
                    nc.tensor.matmul(
                        sc[0:128, cm["sm3"][j]:cm["sm3"][j] + 20],
                        lhsT=k_sb[0:33, kb + 288:kb + 416], rhs=qs,
                        start=True, stop=True)
                probs = probsp.tile([128, NCOL], FP16)
                nc.scalar.activation(probs[0:128, 0:cm["ncol"]],
                                     sc[0:128, 0:cm["ncol"]],
                                     mybir.ActivationFunctionType.Exp,
                                     scale=SCALE)
                return probs

            def do_pv(probs, v_sb, g):
                cm = CM0 if g == 0 else CM
                pv = pvp.tile([128, 12, VA], F32)
                for j in range(4):
                    t = 4 * g + j
                    sb = 32 * j
                    st = slice(sb, sb + 32)
                    has0 = cm["img0"][j] is not None
                    for sub, off, w in ((0, 0, 128), (1, 128, 68)):
                        if has0:
                            nc.tensor.matmul(
                                pv[0:w, 3 * j + sub, :],
                                lhsT=probs[0:128, cm["img0"][j] + off:
                                           cm["img0"][j] + off + w],
                                rhs=v_sb[0:128, t, 0, :],
                                start=True, stop=False)
                        nc.tensor.matmul(
                            pv[0:w, 3 * j + sub, :],
                            lhsT=probs[st, cm["img1"] + off:cm["img1"] + off + w],
                            rhs=v_sb[st, t, 1, :],
                            start=not has0, stop=True, tile_position=(sb, 0))
                    chunks = [(cm["sm1"], 1, st, (sb, 0)),
                              (cm["sm2"][j], 2, slice(0, 128), None),
                              (cm["sm3"][j], 3, slice(0, 128), None)]
                    if has0:
                        chunks.insert(0, (cm["sm0"][j], 0, slice(0, 128), None))
                    for ci, (col, vc, kpart, tp) in enumerate(chunks):
                        nc.tensor.matmul(
                            pv[0:20, 3 * j + 2, :],
                            lhsT=probs[kpart, col:col + 20],
                            rhs=v_sb[kpart, t, vc, :],
                            start=(ci == 0), stop=(ci == len(chunks) - 1),
                            tile_position=tp)

                return pv

            def do_norm(pv, outs, g, fine=False):
                o0, o1, o2 = outs
                recips = recipsp.tile([128, 112], F32)
                pvf = pv[:].rearrange("p g x -> p (g x)")
                nj = 4 if fine else 1
                step = 4 // nj
                for jj in range(nj):
                    c0 = 99 * step * jj
                    nc.vector.reciprocal(
                        recips[0:128, 4 * jj:4 * jj + step],
                        _str2(pvf[0:128, c0 + 32:c0 + 33], (99, step), (1, 1)))
                    nc.vector.reciprocal(
                        recips[0:68, 48 + 4 * jj:48 + 4 * jj + step],
                        _str2(pvf[0:68, c0 + 65:c0 + 66], (99, step), (1, 1)))
                    nc.vector.reciprocal(
                        recips[0:20, 96 + 4 * jj:96 + 4 * jj + step],
                        _str2(pvf[0:20, c0 + 98:c0 + 99], (99, step), (1, 1)))
                    for (ot, phi, sub) in ((o0, 128, 0), (o1, 68, 1), (o2, 20, 2)):
                        num_v = _str2(pvf[0:phi, c0 + 33 * sub:c0 + 33 * sub + 1],
                                      (99, step), (1, 32))
                        rec_b = _str2(recips[0:phi, 48 * sub + 4 * jj:
                                             48 * sub + 4 * jj + 1],
                                      (1, step), (0, 32))
                        out_v = _str2(ot[0:phi, 4 * g + step * jj, 0:1],
                                      (D, step), (1, 32))
                        nc.vector.tensor_mul(out_v, num_v, rec_b)

            # warm the Exp activation table while the first loads run
            scratch = qp.tile([1, 4], F32)
            nc.gpsimd.memset(scratch[:], 0.0)
            nc.scalar.activation(scratch[:], scratch[:],
                                 mybir.ActivationFunctionType.Exp, scale=1.0)

            K4 = 4 * KBLK
            pending = None   # (probs, v_sb, outs, g, i_done)
            for i in range(n_bh):
                q_sb = qp.tile([33, S], FP16)
                k_sb = kp.tile([33, T * KBLK + 60], FP16)
                v_sb = vp.tile([128, T, 4, VA], FP16)
                kc = [0, K4 + 60, 2 * K4 + 60, 3 * K4 + 60, 4 * K4 + 60]
                for g in range(4):
                    if i == 0 and g == 0:
                        # first chunk: parallelize across SP/Pool/ACT queues
                        half = (kc[1] // 2) & ~1
                        nc.sync.dma_start(out=k_sb[:, 0:half],
                                          in_=kall_d[0, :, 0:half])
                        nc.gpsimd.dma_start(out=k_sb[:, half:kc[1]],
                                            in_=kall_d[0, :, half:kc[1]])
                        nc.scalar.dma_start(out=q_sb[:, 0:864],
                                            in_=qaug_d[0, :, 0:864])
                    else:
                        nc.sync.dma_start(out=k_sb[:, kc[g]:kc[g + 1]],
                                          in_=kall_d[i, :, kc[g]:kc[g + 1]])
                        nc.gpsimd.dma_start(
                            out=q_sb[:, 864 * g:864 * (g + 1)],
                            in_=qaug_d[i, :, 864 * g:864 * (g + 1)])
                    if g % 2 == 0:
                        nc.gpsimd.dma_start(
                            out=v_sb[:, 4 * g:4 * g + 8, :, :],
                            in_=vall_d[i, :, 4 * g:4 * g + 8, :, :])
                o0 = outsbp.tile([128, T, D], FP16)
                o1 = outsbp.tile([68, T, D], FP16)
                o2 = outsbp.tile([20, T, D], FP16)

                for g in range(NGRP):
                    probs = do_qk_exp(q_sb, k_sb, g)
                    if pending is not None:
                        pvt = do_pv(pending[0], pending[1], pending[3])
                        do_norm(pvt, pending[2], pending[3])
                        if pending[4] is not None:
                            st_i, st_o = pending[4]
                            nc.sync.dma_start(out=out_d0[st_i], in_=st_o[0][:])
                            nc.gpsimd.dma_start(out=out_d1[st_i], in_=st_o[1][:])
                            nc.gpsimd.dma_start(out=out_d2[st_i], in_=st_o[2][:])
                    pending = (probs, v_sb, (o0, o1, o2), g,
                               (i, (o0, o1, o2)) if g == NGRP - 1 else None)

            pvt = do_pv(pending[0], pending[1], pending[3])
            do_norm(pvt, pending[2], pending[3])
            st_i, st_o = pending[4]
            nc.sync.dma_start(out=out_d0[st_i], in_=st_o[0][:])
            nc.gpsimd.dma_start(out=out_d1[st_i], in_=st_o[1][:])
            nc.scalar.dma_start(out=out_d2[st_i], in_=st_o[2][:])
    nc.compile()
    return nc


_NC = None


def _get_nc():
    global _NC
    if _NC is None:
        _NC = build_nc()
    return _NC


# ---------------------------------------------------------------- entry point

def kernel(q, k, v, feats_per_t, window_len, act_size, img_feat_size):
    assert int(feats_per_t) == F and int(window_len) == W
    assert int(act_size) == 16 and int(img_feat_size) == 196

    packed = _pack_all(np.asarray(q, np.float32), np.asarray(k, np.float32),
                       np.asarray(v, np.float32))
    in_maps = []
    for core in range(N_CORES):
        s = slice(BH_PER_CORE * core, BH_PER_CORE * (core + 1))
        in_maps.append({n: np.ascontiguousarray(a[s])
                        for n, a in packed.items()})

    nc = _get_nc()
    res = run_bass_kernel_spmd(nc, in_maps, list(range(N_CORES)))
    out = np.empty((B * H, S, D), np.float32)
    for core in range(N_CORES):
        r = res.results[core]
        o = _unpack_out(r["out0"].astype(np.float32),
                        r["out1"].astype(np.float32),
                        r["out2"].astype(np.float32))
        out[BH_PER_CORE * core:BH_PER_CORE * (core + 1)] = o
    return out.reshape(B, H, S, D)
